# revision 14
# baseline (speedup 1.0000x reference)
"""Trainium2 Bass kernel for single-head attention (B=4, S=4096, C=D=512).

Sharding: 8 cores = 4 batches x 2 query-halves. Each core:
  - receives xT ([C, S], host-pre-transposed, bf16) ROLLED so its query
    half occupies columns 0..2047 (attention over keys is order-invariant,
    so rolling keys is exact),
  - projects K/V for the full 4096 keys (duplicated across the two cores
    of a batch pair; a pair AllGather was measured at ~112us of rendezvous
    latency on this runtime, far more than the ~30us of duplicate work),
  - projects Q for columns 0..2047 straight from the resident xT,
  - computes softmax(Q K^T / sqrt(D)) V and the output projection.

On-chip layout notes:
  - x is transposed on the HOST and passed bf16 as xT[c, s], so the
    projection matmuls consume it directly: no PE transposes, no
    PSUM->SBUF transpose copies, and the DMA'd tile is itself the
    persistent operand (32KB/partition).
  - Weights are cast to bf16 on the host and DMA'd straight into their
    persistent SBUF tiles (no on-chip f32->bf16 casts).
  - bk drops out (a per-query constant shift of scores cancels in
    softmax); bv+bo fold into a host-side bo_eff added after gather
    (biases are spec'd zero, and the host add is exact regardless), so
    the PE spends zero matmuls on bias work. bq stays on-chip, fused
    into the qT copy's bias port (free).
  - rg0's xT/Wk DMAs are split per 128-row chunk across both HWDGE
    queues and rg0's K-projection is issued dc-outer, so the first
    matmul needs only 256KB of DMA traffic.
  - Scores are computed transposed (scoreT[s, q]) so exp(scoreT) feeds
    the attnT matmul directly with no per-tile transposes.
  - Row sums l[q] are accumulated on the DVE (l_sb += pT per key tile),
    freeing ~27us of PE time vs a ones-matmul per key tile; 1/l is
    applied per query row via a per-partition scale AP.
  - PSUM->SBUF copies for kT/vv/qT run on the Scalar engine (otherwise
    idle) so the DVE never becomes the bottleneck.
  - The s-loop is software-pipelined: score matmuls for key-tile st+1/st+2
    are issued before the exp(st)-consuming matmuls so the in-order PE
    never waits on the ScalarE.
  - The output projection is rt-outer so po[rt] completes early and its
    scaled copy + output DMA (alternating queues) overlap the remaining
    matmuls, shortening the kernel tail.
"""

import sys

for _p in ("/opt/trn_rl_repo", "/root/.axon_site/_ro/trn_rl_repo"):
    if _p not in sys.path:
        sys.path.append(_p)

import numpy as np
import ml_dtypes
import concourse.bacc as bacc
import concourse.mybir as mybir
import concourse.tile as tile
from concourse.bass_utils import run_bass_kernel_spmd

F32 = mybir.dt.float32
BF16 = mybir.dt.bfloat16

MM_DT = BF16

B, S, C, D = 4, 4096, 512, 512
Q = S // 2          # queries per core
N_CORES = 8
SCALE = float(D) ** -0.5
QB = 512            # query block (psum bank width in fp32)
N_QB = Q // QB      # 4 query blocks per core
N_ST = S // 128     # 32 key tiles
N_DC = C // 128     # 4 contraction chunks
N_RG = S // 512     # 8 row groups


def _build_program():
    nc = bacc.Bacc(None, target_bir_lowering=False, debug=False)

    # host-transposed AND host-tiled: x[rg, dc, p, s] = xT[dc*128+p, rg*512+s],
    # so every (rg, dc) chunk is one dense 128KB DMA
    x = nc.dram_tensor("x", [N_RG, N_DC, 128, 512], BF16, kind="ExternalInput")
    w_dram = {
        name: nc.dram_tensor(name, [C, D], BF16, kind="ExternalInput")
        for name in ("Wq", "Wk", "Wv", "Wo")
    }
    bq_dram = nc.dram_tensor("bq", [D], F32, kind="ExternalInput")
    out = nc.dram_tensor("out", [Q, D], F32, kind="ExternalOutput")

    ActFn = mybir.ActivationFunctionType

    with tile.TileContext(nc) as tc:
        persist = tc.alloc_tile_pool(name="persist", bufs=1)
        const = tc.alloc_tile_pool(name="const", bufs=1)
        wkv_pool = tc.alloc_tile_pool(name="wkv", bufs=1)

        ones_f32 = const.tile([128, 128], F32, tag="ones_f32")
        nc.vector.memset(ones_f32[:], 1.0)

        wts = {}

        def emit_weight(name, engine):
            # bf16 weights DMA'd straight into the persistent tile, one
            # DMA per 128-row contraction chunk (fine-grained deps).
            pool = wkv_pool if name in ("Wk", "Wv") else persist
            wt = pool.tile([128, N_DC, D], MM_DT, tag=f"w_{name}", name=f"w_{name}")
            for dc in range(N_DC):
                engine.dma_start(wt[:, dc, :], w_dram[name][dc * 128 : (dc + 1) * 128, :])
            wts[name] = wt

        # ---- persistent activations ----
        kT = persist.tile([128, N_DC, S], MM_DT, tag="kT")     # kT[p, dc, s] = K[s, dc*128+p]
        vv = persist.tile([128, N_ST, D], MM_DT, tag="v")      # vv[p, i, e] = V[i*128+p, e]
        xT = persist.tile([128, N_DC, S], MM_DT, tag="xT")     # xT[p, dc, s] = x[s, dc*128+p]

        # ================= phase A: K/V projections =================
        ps_proj = tc.alloc_tile_pool(name="ps_proj", bufs=4, space="PSUM")

        def emit_xdma(rg):
            # dense 128KB chunks on the sync HWDGE queue (scalar's queue
            # carries the weights so x/W stream in parallel)
            for dc in range(N_DC):
                nc.sync.dma_start(xT[:, dc, rg * 512 : (rg + 1) * 512], x[rg, dc])

        # rg0 x chunks on sync, Wk chunks on scalar: the two queues deliver
        # the dc-k chunk pairs in lockstep with rg0's dc-outer matmuls.
        emit_xdma(0)
        emit_weight("Wk", nc.scalar)
        emit_xdma(1)
        emit_weight("Wv", nc.scalar)

        # pre-load the ACT exp table during phase-A DMA waits (after the
        # Wk/Wv queue programming: ACT_TABLE_LOAD costs 1.3us of ScalarE)
        warm = const.tile([1, 1], F32, tag="warm")
        nc.scalar.activation(warm[:], ones_f32[0:1, 0:1], ActFn.Exp, scale=1.0)

        ones_r = const.tile([128, 128], MM_DT, tag="ones_r")
        nc.vector.tensor_copy(ones_r[:], ones_f32[:])
        bqT = const.tile([128, N_DC], F32, tag="bqT")

        for rg in range(N_RG):              # 8 row groups of 512 rows
            # kT for these 512 rows (bk cancels in softmax: pure matmul).
            # rg0 runs dc-outer so the first matmuls need only the dc0 DMAs.
            pk = [ps_proj.tile([128, 512], F32, tag="pk", name=f"pk{rg}_{g}")
                  for g in range(N_DC)]
            if rg == 0:
                for dc in range(N_DC):
                    for g in range(N_DC):
                        nc.tensor.matmul(pk[g][:], wts["Wk"][:, dc, g * 128 : (g + 1) * 128],
                                         xT[:, dc, rg * 512 : (rg + 1) * 512],
                                         start=(dc == 0), stop=(dc == N_DC - 1))
            else:
                for g in range(N_DC):
                    for dc in range(N_DC):
                        nc.tensor.matmul(pk[g][:], wts["Wk"][:, dc, g * 128 : (g + 1) * 128],
                                         xT[:, dc, rg * 512 : (rg + 1) * 512],
                                         start=(dc == 0), stop=(dc == N_DC - 1))
            for g in range(N_DC):
                nc.scalar.activation(kT[:, g, rg * 512 : (rg + 1) * 512], pk[g][:],
                                     ActFn.Copy)
            # V for these 512 rows (bv folded into bo_eff on the host).
            # rg0 runs dc-outer, chasing the Wv chunk DMAs.
            pv = [ps_proj.tile([128, 512], F32, tag="pk", name=f"pv{rg}_{rt}")
                  for rt in range(4)]
            if rg == 0:
                for dc in range(N_DC):
                    for rt in range(4):
                        nc.tensor.matmul(pv[rt][:], xT[:, dc, rg * 512 + rt * 128 : rg * 512 + (rt + 1) * 128],
                                         wts["Wv"][:, dc, :], start=(dc == 0), stop=(dc == N_DC - 1))
            else:
                for rt in range(4):
                    for dc in range(N_DC):
                        nc.tensor.matmul(pv[rt][:], xT[:, dc, rg * 512 + rt * 128 : rg * 512 + (rt + 1) * 128],
                                         wts["Wv"][:, dc, :], start=(dc == 0), stop=(dc == N_DC - 1))
            for rt in range(4):
                nc.scalar.activation(vv[:, rg * 4 + rt, :], pv[rt][:], ActFn.Copy)
            # queue upcoming x row groups / weights while rg's matmuls run
            if rg == 0:
                emit_xdma(2)
                emit_xdma(3)
            elif rg == 1:
                emit_xdma(4)
                emit_xdma(5)
            elif rg == 2:
                # Wq/Wo on the gpsimd SWDGE queue (~43GB/s): needed only at
                # phase-B start ~35us later, and this keeps both HWDGE
                # queues clear for x and the kT/vv copies on scalar
                emit_weight("Wq", nc.gpsimd)
                emit_weight("Wo", nc.gpsimd)
                emit_xdma(6)
                emit_xdma(7)
            elif rg == 5:
                # tiny 4B-element bias DMAs, needed only at phase-B start
                for g in range(N_DC):
                    nc.gpsimd.dma_start(bqT[:, g : g + 1],
                                        bq_dram[g * 128 : (g + 1) * 128].unsqueeze(1))

        ps_proj.release()
        wkv_pool.release()

        # ================= phase B: attention =================
        with tc.tile_pool(name="qT", bufs=2) as qTp, \
             tc.tile_pool(name="pT", bufs=8) as pTp, \
             tc.tile_pool(name="rl", bufs=2) as rlp, \
             tc.tile_pool(name="attnT", bufs=2) as attnTp, \
             tc.tile_pool(name="osb", bufs=4) as osbp, \
             tc.tile_pool(name="ps_at", bufs=4, space="PSUM") as ps_atp, \
             tc.tile_pool(name="ps_s", bufs=3, space="PSUM") as ps_sp, \
             tc.tile_pool(name="ps_l", bufs=1, space="PSUM") as ps_lp:

            def emit_qproj(qb):
                # Q projection for one 512-query block, straight from the
                # resident xT columns [qb*512, (qb+1)*512).
                qT = qTp.tile([128, N_DC, 512], MM_DT, tag="qT", name=f"qT{qb}")
                for g in range(N_DC):
                    pq = ps_sp.tile([128, 512], F32, tag="ss", name=f"pq{qb}_{g}")
                    for dc in range(N_DC):
                        nc.tensor.matmul(pq[:], wts["Wq"][:, dc, g * 128 : (g + 1) * 128],
                                         xT[:, dc, qb * 512 : (qb + 1) * 512],
                                         start=(dc == 0), stop=(dc == N_DC - 1))
                    nc.scalar.activation(qT[:, g, :], pq[:], ActFn.Identity,
                                         bias=bqT[:, g : g + 1])
                return qT

            def emit_score(qb, st, qT):
                ss = ps_sp.tile([128, 512], F32, tag="ss", name=f"ss{qb}_{st}")
                for dc in range(N_DC):
                    nc.tensor.matmul(ss[:], kT[:, dc, st * 128 : (st + 1) * 128],
                                     qT[:, dc, :], start=(dc == 0), stop=(dc == N_DC - 1))
                return ss

            qT_cur = emit_qproj(0)
            for qb in range(N_QB):
                qT = qT_cur
                l_sb = rlp.tile([128, 512], F32, tag="l_sb", name=f"lsb{qb}")
                at_ps = [ps_atp.tile([128, 512], F32, tag="at", name=f"at{qb}_{et}")
                         for et in range(4)]
                ss_q = [emit_score(qb, 0, qT), emit_score(qb, 1, qT)]
                for st in range(N_ST):
                    if st + 2 < N_ST:
                        ss_q.append(emit_score(qb, st + 2, qT))
                    ss = ss_q.pop(0)
                    pT = pTp.tile([128, 512], MM_DT, tag="pT", name=f"pT{qb}_{st}")
                    nc.scalar.activation(pT[:], ss[:], ActFn.Exp, scale=SCALE)
                    for et in range(4):
                        nc.tensor.matmul(at_ps[et][:], vv[:, st, et * 128 : (et + 1) * 128],
                                         pT[:], start=(st == 0), stop=(st == N_ST - 1))
                    # row-sum accumulation on the DVE (off the PE)
                    if st == 0:
                        nc.vector.tensor_copy(l_sb[:], pT[:])
                    else:
                        nc.vector.tensor_add(l_sb[:], l_sb[:], pT[:])

                if qb + 1 < N_QB:
                    qT_cur = emit_qproj(qb + 1)

                # --- epilogue: 1/l arranged with queries on partitions
                # ([128,4] via tiny transposing matmuls -> fast reciprocal),
                # applied per query row by a per-partition scale AP ---
                lbf = rlp.tile([128, 512], MM_DT, tag="lbf", name=f"lbf{qb}")
                nc.vector.tensor_copy(lbf[:], l_sb[:])
                l_ps = ps_lp.tile([128, 512], F32, tag="l", name=f"l{qb}")
                nc.tensor.matmul(l_ps[:], ones_r[:], lbf[:])
                l_row = rlp.tile([1, 512], F32, tag="l_row", name=f"lrow{qb}")
                nc.vector.tensor_copy(l_row[:], l_ps[0:1, :])
                lt_ps = ps_lp.tile([128, 4], F32, tag="l", name=f"lt{qb}")
                for rt in range(4):
                    nc.tensor.matmul(lt_ps[:, rt : rt + 1],
                                     l_row[0:1, rt * 128 : (rt + 1) * 128],
                                     ones_f32[0:1, 0:1])
                rlT = rlp.tile([128, 4], F32, tag="rlT", name=f"rlT{qb}")
                nc.vector.reciprocal(rlT[:], lt_ps[:])

                attnT = attnTp.tile([128, 4, 512], MM_DT, tag="attnT", name=f"attnT{qb}")
                for et in range(4):
                    nc.vector.tensor_copy(attnT[:, et, :], at_ps[et][:])
                # rt-outer: po[rt] completes after its own 4 matmuls, so the
                # scaled copy + output DMA for rt=0 overlap rt=1..3's matmuls
                # (shortens the kernel tail after the last qb)
                last = qb == N_QB - 1
                for rt in range(4):
                    po = ps_atp.tile([128, 512], F32, tag="at", name=f"po{qb}_{rt}")
                    for ec in range(4):
                        nc.tensor.matmul(po[:], attnT[:, ec, rt * 128 : (rt + 1) * 128],
                                         wts["Wo"][:, ec, :], start=(ec == 0), stop=(ec == 3))
                    ot = osbp.tile([128, D], F32, tag="ot", name=f"ot{qb}_{rt}")
                    # ot = po * (1/l[row]): DVE (ScalarE is busy with the next
                    # block's qT copies + exps).  For the final block there is
                    # no next block, so split the work across Scalar+Vector
                    # and rotate DMA queues to shorten the kernel tail.
                    if last:
                        nc.vector.tensor_scalar_mul(ot[:, 0:256], po[:, 0:256],
                                                    rlT[:, rt : rt + 1])
                        nc.scalar.activation(ot[:, 256:512], po[:, 256:512],
                                             ActFn.Copy, scale=rlT[:, rt : rt + 1])
                        row = out[(qb * 4 + rt) * 128 : (qb * 4 + rt + 1) * 128, :]
                        nc.sync.dma_start(row[:, 0:256], ot[:, 0:256])
                        nc.scalar.dma_start(row[:, 256:512], ot[:, 256:512])
                    else:
                        nc.vector.tensor_scalar_mul(ot[:], po[:], rlT[:, rt : rt + 1])
                        eng = nc.sync if rt % 2 == 0 else nc.scalar
                        eng.dma_start(out[(qb * 4 + rt) * 128 : (qb * 4 + rt + 1) * 128, :], ot[:])

        const.release()
        persist.release()

    nc.compile()
    return nc


_NC_CACHE = None


def _get_nc():
    global _NC_CACHE
    if _NC_CACHE is None:
        _NC_CACHE = _build_program()
    return _NC_CACHE


def kernel(**inputs):
    x = np.asarray(inputs["x"], dtype=np.float32)
    # host-side transpose to xT[c, s] per batch, cast bf16
    xt = np.ascontiguousarray(
        x.reshape(B, S, C).transpose(0, 2, 1)).astype(ml_dtypes.bfloat16)

    def tile_x(xb):
        # xT[c, s] -> [rg, dc, p, s] so each (rg, dc) chunk is contiguous
        return np.ascontiguousarray(
            xb.reshape(N_DC, 128, N_RG, 512).transpose(2, 0, 1, 3))
    ws = {k: np.asarray(inputs[k], dtype=np.float32).astype(ml_dtypes.bfloat16)
          for k in ("Wq", "Wk", "Wv", "Wo")}
    bq = np.ascontiguousarray(np.asarray(inputs["bq"], dtype=np.float32))
    # bv shifts every attention output row by a constant, so it folds into
    # the output bias: out = attn@Wo + (bo + bv@Wo), added on the host
    # after gather.  bk cancels in softmax.
    bo_eff = (np.asarray(inputs["bo"], dtype=np.float32)
              + np.asarray(inputs["bv"], dtype=np.float32)
              @ np.asarray(inputs["Wo"], dtype=np.float32))

    in_maps = []
    for c in range(N_CORES):
        b, h = divmod(c, 2)
        xb = xt[b]
        if h:
            # roll keys so this core's query half occupies columns 0..2047;
            # attention over keys is order-invariant so this is exact.
            xb = np.concatenate([xb[:, Q:], xb[:, :Q]], axis=1)
        m = {"x": tile_x(xb), "bq": bq}
        m.update(ws)
        in_maps.append(m)

    nc = _get_nc()
    try:
        res = run_bass_kernel_spmd(nc, in_maps, core_ids=list(range(N_CORES)))
    except Exception:
        # transient NRT/device hiccups recover on retry
        import time
        time.sleep(15)
        res = run_bass_kernel_spmd(nc, in_maps, core_ids=list(range(N_CORES)))

    out = np.empty((B, S, D), dtype=np.float32)
    for c in range(N_CORES):
        b, h = divmod(c, 2)
        out[b, h * Q : (h + 1) * Q] = res.results[c]["out"]
    if np.any(bo_eff):
        out += bo_eff
    return out.reshape(B, 64, 64, D)


# revision 15
# speedup vs baseline: 1.1411x; 1.1411x over previous
"""Trainium2 Bass kernel for single-head attention (B=4, S=4096, C=D=512).

Sharding: 8 cores = 4 batches x 2 query-halves. Each core receives xT
([C, S], host-pre-transposed bf16) ROLLED so its query half occupies
columns 0..2047 (attention over keys is order-invariant, so rolling keys
is exact).

The key trick: both weight pairs fold on the host, eliminating two of the
four projection stages on-chip (exact algebra, not an approximation):

  score_qk = (x_q Wq + bq)(x_k Wk + bk)^T
           = [x_q (Wq Wk^T) + bq Wk^T] x_k^T + const(q)   [cancels in softmax]
  out      = (P/l)(x Wv + bv) Wo + bo
           = (P/l) x (Wv Wo) + (bv Wo + bo)

With M = Wq Wk^T and N = Wv Wo precomputed f32 on the host:
  - NO K projection and NO kT tile: score matmuls contract q~ = x M + bq Wk^T
    directly against the resident xT chunks,
  - NO output projection: the attention accumulation P^T-slices x (x N)
    is issued with lhsT = pT column slices so PSUM accumulates at[q, d],
    already output-oriented; out rows = at * (1/l) + (bv Wo + bo),
  - the bias fold keeps everything exact: bq enters via the q~ copy's
    bias port (b~ = bq Wk^T), bk cancels in softmax, bv+bo are added on
    the host after gather.

Per-core PE work drops from ~302us to ~261us of matmul streaming.

On-chip layout notes:
  - x is transposed AND tiled on the HOST (x[rg, dc, p, s]) so every
    (rg, dc) chunk is one dense 128KB DMA and the DMA'd tile is itself
    the persistent matmul operand for V~ projection, scores, and q~.
  - M/N are bf16 from the host, DMA'd straight into persistent tiles.
  - Scores are computed transposed (scoreT[s, q]) so exp(scoreT) feeds
    the attention matmul directly with no per-tile transposes.
  - Row sums l[q] accumulate on the DVE (l_sb += pT per key tile); 1/l
    is applied per query row via a per-partition scale AP.
  - PSUM->SBUF copies for vv/qT run on the Scalar engine.
  - The s-loop is software-pipelined: score matmuls for key-tile st+1/st+2
    are issued before the exp(st)-consuming matmuls so the in-order PE
    never waits on the ScalarE.
"""

import sys

for _p in ("/opt/trn_rl_repo", "/root/.axon_site/_ro/trn_rl_repo"):
    if _p not in sys.path:
        sys.path.append(_p)

import numpy as np
import ml_dtypes
import concourse.bacc as bacc
import concourse.mybir as mybir
import concourse.tile as tile
from concourse.bass_utils import run_bass_kernel_spmd

F32 = mybir.dt.float32
BF16 = mybir.dt.bfloat16

MM_DT = BF16

B, S, C, D = 4, 4096, 512, 512
Q = S // 2          # queries per core
N_CORES = 8
SCALE = float(D) ** -0.5
QB = 512            # query block (psum bank width in fp32)
N_QB = Q // QB      # 4 query blocks per core
N_ST = S // 128     # 32 key tiles
N_DC = C // 128     # 4 contraction chunks
N_RG = S // 512     # 8 row groups


def _build_program():
    nc = bacc.Bacc(None, target_bir_lowering=False, debug=False)

    # host-transposed AND host-tiled: x[rg, dc, p, s] = xT[dc*128+p, rg*512+s],
    # so every (rg, dc) chunk is one dense 128KB DMA
    x = nc.dram_tensor("x", [N_RG, N_DC, 128, 512], BF16, kind="ExternalInput")
    w_dram = {
        name: nc.dram_tensor(name, [C, D], BF16, kind="ExternalInput")
        for name in ("M", "N")
    }
    bq_dram = nc.dram_tensor("bq", [D], F32, kind="ExternalInput")  # bq Wk^T
    out = nc.dram_tensor("out", [Q, D], F32, kind="ExternalOutput")

    ActFn = mybir.ActivationFunctionType

    with tile.TileContext(nc) as tc:
        persist = tc.alloc_tile_pool(name="persist", bufs=1)
        const = tc.alloc_tile_pool(name="const", bufs=1)

        ones_f32 = const.tile([128, 128], F32, tag="ones_f32")
        nc.vector.memset(ones_f32[:], 1.0)

        wts = {}

        def emit_weight(name, engine):
            wt = persist.tile([128, N_DC, D], MM_DT, tag=f"w_{name}", name=f"w_{name}")
            for dc in range(N_DC):
                engine.dma_start(wt[:, dc, :], w_dram[name][dc * 128 : (dc + 1) * 128, :])
            wts[name] = wt

        # ---- persistent activations ----
        vv = persist.tile([128, N_ST, D], MM_DT, tag="v")   # vv[p, i, e] = (x N)[i*128+p, e]
        xT = persist.tile([128, N_DC, S], MM_DT, tag="xT")  # xT[p, dc, s] = x[s, dc*128+p]

        # ================= phase A: V~ = x N projection =================
        ps_proj = tc.alloc_tile_pool(name="ps_proj", bufs=4, space="PSUM")

        def emit_xdma(rg, eng):
            for dc in range(N_DC):
                eng.dma_start(xT[:, dc, rg * 512 : (rg + 1) * 512], x[rg, dc])

        # rg0/rg1 x chunks on sync, N chunks on scalar: the two queues
        # deliver the dc-k chunk pairs in lockstep with rg0's dc-outer
        # matmuls; M (needed only at phase-B start) follows on scalar.
        emit_xdma(0, nc.sync)
        emit_weight("N", nc.scalar)
        emit_xdma(1, nc.sync)

        warm = const.tile([1, 1], F32, tag="warm")
        nc.scalar.activation(warm[:], ones_f32[0:1, 0:1], ActFn.Exp, scale=1.0)

        ones_r = const.tile([128, 128], MM_DT, tag="ones_r")
        nc.vector.tensor_copy(ones_r[:], ones_f32[:])
        bqT = const.tile([128, N_DC], F32, tag="bqT")

        for rg in range(N_RG):              # 8 row groups of 512 rows
            # V~ for these 512 rows.  rg0 runs dc-outer, chasing the N DMAs.
            pv = [ps_proj.tile([128, 512], F32, tag="pv", name=f"pv{rg}_{rt}")
                  for rt in range(4)]
            if rg == 0:
                for dc in range(N_DC):
                    for rt in range(4):
                        nc.tensor.matmul(pv[rt][:], xT[:, dc, rg * 512 + rt * 128 : rg * 512 + (rt + 1) * 128],
                                         wts["N"][:, dc, :], start=(dc == 0), stop=(dc == N_DC - 1))
            else:
                for rt in range(4):
                    for dc in range(N_DC):
                        nc.tensor.matmul(pv[rt][:], xT[:, dc, rg * 512 + rt * 128 : rg * 512 + (rt + 1) * 128],
                                         wts["N"][:, dc, :], start=(dc == 0), stop=(dc == N_DC - 1))
            for rt in range(4):
                nc.scalar.activation(vv[:, rg * 4 + rt, :], pv[rt][:], ActFn.Copy)
            # queue upcoming x row groups / weights while rg's matmuls run
            if rg == 0:
                emit_xdma(2, nc.scalar)
                emit_xdma(3, nc.sync)
            elif rg == 1:
                emit_weight("M", nc.scalar)
                emit_xdma(4, nc.sync)
            elif rg == 2:
                emit_xdma(5, nc.scalar)
                emit_xdma(6, nc.sync)
                emit_xdma(7, nc.scalar)
            elif rg == 4:
                # tiny 4B-element bias DMAs, needed only at phase-B start
                for g in range(N_DC):
                    nc.gpsimd.dma_start(bqT[:, g : g + 1],
                                        bq_dram[g * 128 : (g + 1) * 128].unsqueeze(1))

        ps_proj.release()

        # ================= phase B: attention =================
        with tc.tile_pool(name="qT", bufs=2) as qTp, \
             tc.tile_pool(name="pT", bufs=8) as pTp, \
             tc.tile_pool(name="rl", bufs=2) as rlp, \
             tc.tile_pool(name="osb", bufs=4) as osbp, \
             tc.tile_pool(name="ps_at", bufs=4, space="PSUM") as ps_atp, \
             tc.tile_pool(name="ps_s", bufs=3, space="PSUM") as ps_sp, \
             tc.tile_pool(name="ps_l", bufs=1, space="PSUM") as ps_lp:

            def emit_qproj(qb):
                # q~ = x M + bq Wk^T for one 512-query block, straight from
                # the resident xT columns [qb*512, (qb+1)*512).
                qT = qTp.tile([128, N_DC, 512], MM_DT, tag="qT", name=f"qT{qb}")
                for g in range(N_DC):
                    pq = ps_sp.tile([128, 512], F32, tag="ss", name=f"pq{qb}_{g}")
                    for dc in range(N_DC):
                        nc.tensor.matmul(pq[:], wts["M"][:, dc, g * 128 : (g + 1) * 128],
                                         xT[:, dc, qb * 512 : (qb + 1) * 512],
                                         start=(dc == 0), stop=(dc == N_DC - 1))
                    nc.scalar.activation(qT[:, g, :], pq[:], ActFn.Identity,
                                         bias=bqT[:, g : g + 1])
                return qT

            def emit_score(qb, st, qT):
                # scoreT[s in st, q] = sum_dc xT[:, dc, st]^T qT[:, dc, :]
                ss = ps_sp.tile([128, 512], F32, tag="ss", name=f"ss{qb}_{st}")
                for dc in range(N_DC):
                    nc.tensor.matmul(ss[:], xT[:, dc, st * 128 : (st + 1) * 128],
                                     qT[:, dc, :], start=(dc == 0), stop=(dc == N_DC - 1))
                return ss

            qT_cur = emit_qproj(0)
            for qb in range(N_QB):
                qT = qT_cur
                l_sb = rlp.tile([128, 512], F32, tag="l_sb", name=f"lsb{qb}")
                # at[qt][q in block, d] accumulates the UNNORMALIZED output
                # rows for this query block (P x N-projected values)
                at_ps = [ps_atp.tile([128, 512], F32, tag="at", name=f"at{qb}_{qt}")
                         for qt in range(4)]
                ss_q = [emit_score(qb, 0, qT), emit_score(qb, 1, qT)]
                for st in range(N_ST):
                    if st + 2 < N_ST:
                        ss_q.append(emit_score(qb, st + 2, qT))
                    ss = ss_q.pop(0)
                    pT = pTp.tile([128, 512], MM_DT, tag="pT", name=f"pT{qb}_{st}")
                    nc.scalar.activation(pT[:], ss[:], ActFn.Exp, scale=SCALE)
                    for qt in range(4):
                        nc.tensor.matmul(at_ps[qt][:], pT[:, qt * 128 : (qt + 1) * 128],
                                         vv[:, st, :], start=(st == 0), stop=(st == N_ST - 1))
                    # row-sum accumulation on the DVE (off the PE)
                    if st == 0:
                        nc.vector.tensor_copy(l_sb[:], pT[:])
                    else:
                        nc.vector.tensor_add(l_sb[:], l_sb[:], pT[:])

                if qb + 1 < N_QB:
                    qT_cur = emit_qproj(qb + 1)

                # --- epilogue: 1/l arranged with queries on partitions
                # ([128,4] via tiny transposing matmuls -> fast reciprocal) ---
                lbf = rlp.tile([128, 512], MM_DT, tag="lbf", name=f"lbf{qb}")
                nc.vector.tensor_copy(lbf[:], l_sb[:])
                l_ps = ps_lp.tile([128, 512], F32, tag="l", name=f"l{qb}")
                nc.tensor.matmul(l_ps[:], ones_r[:], lbf[:])
                l_row = rlp.tile([1, 512], F32, tag="l_row", name=f"lrow{qb}")
                nc.vector.tensor_copy(l_row[:], l_ps[0:1, :])
                lt_ps = ps_lp.tile([128, 4], F32, tag="l", name=f"lt{qb}")
                for qt in range(4):
                    nc.tensor.matmul(lt_ps[:, qt : qt + 1],
                                     l_row[0:1, qt * 128 : (qt + 1) * 128],
                                     ones_f32[0:1, 0:1])
                rlT = rlp.tile([128, 4], F32, tag="rlT", name=f"rlT{qb}")
                nc.vector.reciprocal(rlT[:], lt_ps[:])

                # out rows = at * (1/l); for the final block split the scaled
                # copies across Vector+Scalar and both DMA queues to shorten
                # the kernel tail
                last = qb == N_QB - 1
                for qt in range(4):
                    ot = osbp.tile([128, D], F32, tag="ot", name=f"ot{qb}_{qt}")
                    row = out[(qb * 4 + qt) * 128 : (qb * 4 + qt + 1) * 128, :]
                    if last:
                        nc.vector.tensor_scalar_mul(ot[:, 0:256], at_ps[qt][:, 0:256],
                                                    rlT[:, qt : qt + 1])
                        nc.scalar.activation(ot[:, 256:512], at_ps[qt][:, 256:512],
                                             ActFn.Copy, scale=rlT[:, qt : qt + 1])
                        nc.sync.dma_start(row[:, 0:256], ot[:, 0:256])
                        nc.scalar.dma_start(row[:, 256:512], ot[:, 256:512])
                    else:
                        nc.vector.tensor_scalar_mul(ot[:], at_ps[qt][:], rlT[:, qt : qt + 1])
                        eng = nc.sync if qt % 2 == 0 else nc.scalar
                        eng.dma_start(row, ot[:])

        const.release()
        persist.release()

    nc.compile()
    return nc


_NC_CACHE = None


def _get_nc():
    global _NC_CACHE
    if _NC_CACHE is None:
        _NC_CACHE = _build_program()
    return _NC_CACHE


def kernel(**inputs):
    f32 = np.float32
    x = np.asarray(inputs["x"], dtype=f32)
    # host-side transpose to xT[c, s] per batch, cast bf16
    xt = np.ascontiguousarray(
        x.reshape(B, S, C).transpose(0, 2, 1)).astype(ml_dtypes.bfloat16)

    def tile_x(xb):
        # xT[c, s] -> [rg, dc, p, s] so each (rg, dc) chunk is contiguous
        return np.ascontiguousarray(
            xb.reshape(N_DC, 128, N_RG, 512).transpose(2, 0, 1, 3))

    Wq = np.asarray(inputs["Wq"], dtype=f32)
    Wk = np.asarray(inputs["Wk"], dtype=f32)
    Wv = np.asarray(inputs["Wv"], dtype=f32)
    Wo = np.asarray(inputs["Wo"], dtype=f32)
    # host-side weight folds (exact algebra, f32):
    #   score = [x (Wq Wk^T) + bq Wk^T] x^T  (+ per-query const, cancels)
    #   out   = (P/l) x (Wv Wo) + (bv Wo + bo)
    M = np.ascontiguousarray(Wq @ Wk.T).astype(ml_dtypes.bfloat16)
    N = np.ascontiguousarray(Wv @ Wo).astype(ml_dtypes.bfloat16)
    bqf = np.ascontiguousarray(np.asarray(inputs["bq"], dtype=f32) @ Wk.T)
    bo_eff = np.asarray(inputs["bo"], dtype=f32) + np.asarray(inputs["bv"], dtype=f32) @ Wo

    in_maps = []
    for c in range(N_CORES):
        b, h = divmod(c, 2)
        xb = xt[b]
        if h:
            # roll keys so this core's query half occupies columns 0..2047;
            # attention over keys is order-invariant so this is exact.
            xb = np.concatenate([xb[:, Q:], xb[:, :Q]], axis=1)
        in_maps.append({"x": tile_x(xb), "bq": bqf, "M": M, "N": N})

    nc = _get_nc()
    try:
        res = run_bass_kernel_spmd(nc, in_maps, core_ids=list(range(N_CORES)))
    except Exception:
        # transient NRT/device hiccups recover on retry
        import time
        time.sleep(15)
        res = run_bass_kernel_spmd(nc, in_maps, core_ids=list(range(N_CORES)))

    out = np.empty((B, S, D), dtype=f32)
    for c in range(N_CORES):
        b, h = divmod(c, 2)
        out[b, h * Q : (h + 1) * Q] = res.results[c]["out"]
    if np.any(bo_eff):
        out += bo_eff
    return out.reshape(B, 64, 64, D)


# revision 17
# speedup vs baseline: 1.1461x; 1.0044x over previous
"""Trainium2 Bass kernel for single-head attention (B=4, S=4096, C=D=512).

Sharding: 8 cores = 4 batches x 2 query-halves. Each core receives xT
([C, S], host-pre-transposed bf16) ROLLED so its query half occupies
columns 0..2047 (attention over keys is order-invariant, so rolling keys
is exact).

The key trick: both weight pairs fold on the host, eliminating two of the
four projection stages on-chip (exact algebra, not an approximation):

  score_qk = (x_q Wq + bq)(x_k Wk + bk)^T
           = [x_q (Wq Wk^T) + bq Wk^T] x_k^T + const(q)   [cancels in softmax]
  out      = (P/l)(x Wv + bv) Wo + bo
           = (P/l) x (Wv Wo) + (bv Wo + bo)

With M = Wq Wk^T and N = Wv Wo precomputed f32 on the host:
  - NO K projection and NO kT tile: score matmuls contract q~ = x M + bq Wk^T
    directly against the resident xT chunks,
  - NO output projection: the attention accumulation P^T-slices x (x N)
    is issued with lhsT = pT column slices so PSUM accumulates at[q, d],
    already output-oriented; out rows = at * (1/l) + (bv Wo + bo),
  - the bias fold keeps everything exact: bq enters via the q~ copy's
    bias port (b~ = bq Wk^T), bk cancels in softmax, bv+bo are added on
    the host after gather.

Per-core PE work drops from ~302us to ~261us of matmul streaming.

On-chip layout notes:
  - x is transposed AND tiled on the HOST (x[rg, dc, p, s]) so every
    (rg, dc) chunk is one dense 128KB DMA and the DMA'd tile is itself
    the persistent matmul operand for V~ projection, scores, and q~.
  - M/N are bf16 from the host, DMA'd straight into persistent tiles.
  - Scores are computed transposed (scoreT[s, q]) so exp(scoreT) feeds
    the attention matmul directly with no per-tile transposes.
  - Row sums l[q] accumulate on the DVE (l_sb += pT per key tile); 1/l
    is applied per query row via a per-partition scale AP.
  - PSUM->SBUF copies for vv/qT run on the Scalar engine.
  - The s-loop is software-pipelined: score matmuls for key-tile st+1/st+2
    are issued before the exp(st)-consuming matmuls so the in-order PE
    never waits on the ScalarE.
"""

import sys

for _p in ("/opt/trn_rl_repo", "/root/.axon_site/_ro/trn_rl_repo"):
    if _p not in sys.path:
        sys.path.append(_p)

import numpy as np
import ml_dtypes
import concourse.bacc as bacc
import concourse.mybir as mybir
import concourse.tile as tile
from concourse.bass_utils import run_bass_kernel_spmd

F32 = mybir.dt.float32
BF16 = mybir.dt.bfloat16

MM_DT = BF16

B, S, C, D = 4, 4096, 512, 512
Q = S // 2          # queries per core
N_CORES = 8
SCALE = float(D) ** -0.5
QB = 512            # query block (psum bank width in fp32)
N_QB = Q // QB      # 4 query blocks per core
N_ST = S // 128     # 32 key tiles
N_DC = C // 128     # 4 contraction chunks
N_RG = S // 512     # 8 row groups


def _build_program():
    nc = bacc.Bacc(None, target_bir_lowering=False, debug=False)

    # host-transposed AND host-tiled: x[rg, dc, p, s] = xT[dc*128+p, rg*512+s],
    # so every (rg, dc) chunk is one dense 128KB DMA
    x = nc.dram_tensor("x", [N_RG, N_DC, 128, 512], BF16, kind="ExternalInput")
    w_dram = {
        name: nc.dram_tensor(name, [C, D], BF16, kind="ExternalInput")
        for name in ("M", "N")
    }
    bq_dram = nc.dram_tensor("bq", [D], F32, kind="ExternalInput")  # bq Wk^T
    out = nc.dram_tensor("out", [Q, D], F32, kind="ExternalOutput")

    ActFn = mybir.ActivationFunctionType

    with tile.TileContext(nc) as tc:
        persist = tc.alloc_tile_pool(name="persist", bufs=1)
        const = tc.alloc_tile_pool(name="const", bufs=1)

        ones_f32 = const.tile([128, 128], F32, tag="ones_f32")
        nc.vector.memset(ones_f32[:], 1.0)

        wts = {}

        def emit_weight(name, engine):
            wt = persist.tile([128, N_DC, D], MM_DT, tag=f"w_{name}", name=f"w_{name}")
            for dc in range(N_DC):
                engine.dma_start(wt[:, dc, :], w_dram[name][dc * 128 : (dc + 1) * 128, :])
            wts[name] = wt

        # ---- persistent activations ----
        vv = persist.tile([128, N_ST, D], MM_DT, tag="v")   # vv[p, i, e] = (x N)[i*128+p, e]
        xT = persist.tile([128, N_DC, S], MM_DT, tag="xT")  # xT[p, dc, s] = x[s, dc*128+p]

        # ================= phase A: V~ = x N projection =================
        ps_proj = tc.alloc_tile_pool(name="ps_proj", bufs=4, space="PSUM")

        def emit_xdma(rg, eng):
            for dc in range(N_DC):
                eng.dma_start(xT[:, dc, rg * 512 : (rg + 1) * 512], x[rg, dc])

        # rg0/rg1 x chunks on sync, N chunks on scalar: the two queues
        # deliver the dc-k chunk pairs in lockstep with rg0's dc-outer
        # matmuls; M (needed only at phase-B start) follows on scalar.
        emit_xdma(0, nc.sync)
        emit_weight("N", nc.scalar)
        emit_xdma(1, nc.sync)

        warm = const.tile([1, 1], F32, tag="warm")
        nc.scalar.activation(warm[:], ones_f32[0:1, 0:1], ActFn.Exp, scale=1.0)

        bqT = const.tile([128, N_DC], F32, tag="bqT")

        for rg in range(N_RG):              # 8 row groups of 512 rows
            # V~ for these 512 rows.  rg0 runs dc-outer, chasing the N DMAs.
            pv = [ps_proj.tile([128, 512], F32, tag="pv", name=f"pv{rg}_{rt}")
                  for rt in range(4)]
            if rg == 0:
                for dc in range(N_DC):
                    for rt in range(4):
                        nc.tensor.matmul(pv[rt][:], xT[:, dc, rg * 512 + rt * 128 : rg * 512 + (rt + 1) * 128],
                                         wts["N"][:, dc, :], start=(dc == 0), stop=(dc == N_DC - 1))
            else:
                for rt in range(4):
                    for dc in range(N_DC):
                        nc.tensor.matmul(pv[rt][:], xT[:, dc, rg * 512 + rt * 128 : rg * 512 + (rt + 1) * 128],
                                         wts["N"][:, dc, :], start=(dc == 0), stop=(dc == N_DC - 1))
            for rt in range(4):
                nc.scalar.activation(vv[:, rg * 4 + rt, :], pv[rt][:], ActFn.Copy)
            # queue upcoming x row groups / weights while rg's matmuls run
            if rg == 0:
                emit_xdma(2, nc.scalar)
                emit_xdma(3, nc.sync)
            elif rg == 1:
                emit_weight("M", nc.scalar)
                emit_xdma(4, nc.sync)
            elif rg == 2:
                emit_xdma(5, nc.scalar)
                emit_xdma(6, nc.sync)
                emit_xdma(7, nc.scalar)
            elif rg == 4:
                # tiny 4B-element bias DMAs, needed only at phase-B start
                for g in range(N_DC):
                    nc.gpsimd.dma_start(bqT[:, g : g + 1],
                                        bq_dram[g * 128 : (g + 1) * 128].unsqueeze(1))

        ps_proj.release()

        # ================= phase B: attention =================
        with tc.tile_pool(name="qT", bufs=2) as qTp, \
             tc.tile_pool(name="pT", bufs=8) as pTp, \
             tc.tile_pool(name="rl", bufs=2) as rlp, \
             tc.tile_pool(name="osb", bufs=4) as osbp, \
             tc.tile_pool(name="ps_at", bufs=4, space="PSUM") as ps_atp, \
             tc.tile_pool(name="ps_s", bufs=3, space="PSUM") as ps_sp, \
             tc.tile_pool(name="ps_l", bufs=1, space="PSUM") as ps_lp:

            def emit_qproj(qb):
                # q~ = x M + bq Wk^T for one 512-query block, straight from
                # the resident xT columns [qb*512, (qb+1)*512).
                qT = qTp.tile([128, N_DC, 512], MM_DT, tag="qT", name=f"qT{qb}")
                for g in range(N_DC):
                    pq = ps_sp.tile([128, 512], F32, tag="ss", name=f"pq{qb}_{g}")
                    for dc in range(N_DC):
                        nc.tensor.matmul(pq[:], wts["M"][:, dc, g * 128 : (g + 1) * 128],
                                         xT[:, dc, qb * 512 : (qb + 1) * 512],
                                         start=(dc == 0), stop=(dc == N_DC - 1))
                    nc.scalar.activation(qT[:, g, :], pq[:], ActFn.Identity,
                                         bias=bqT[:, g : g + 1])
                return qT

            def emit_score(qb, st, qT):
                # scoreT[s in st, q] = sum_dc xT[:, dc, st]^T qT[:, dc, :]
                ss = ps_sp.tile([128, 512], F32, tag="ss", name=f"ss{qb}_{st}")
                for dc in range(N_DC):
                    nc.tensor.matmul(ss[:], xT[:, dc, st * 128 : (st + 1) * 128],
                                     qT[:, dc, :], start=(dc == 0), stop=(dc == N_DC - 1))
                return ss

            qT_cur = emit_qproj(0)
            for qb in range(N_QB):
                qT = qT_cur
                l_sb = rlp.tile([128, 512], F32, tag="l_sb", name=f"lsb{qb}")
                # at[qt][q in block, d] accumulates the UNNORMALIZED output
                # rows for this query block (P x N-projected values)
                at_ps = [ps_atp.tile([128, 512], F32, tag="at", name=f"at{qb}_{qt}")
                         for qt in range(4)]
                ss_q = [emit_score(qb, 0, qT), emit_score(qb, 1, qT)]
                for st in range(N_ST):
                    if st + 2 < N_ST:
                        ss_q.append(emit_score(qb, st + 2, qT))
                    ss = ss_q.pop(0)
                    pT = pTp.tile([128, 512], MM_DT, tag="pT", name=f"pT{qb}_{st}")
                    nc.scalar.activation(pT[:], ss[:], ActFn.Exp, scale=SCALE)
                    for qt in range(4):
                        nc.tensor.matmul(at_ps[qt][:], pT[:, qt * 128 : (qt + 1) * 128],
                                         vv[:, st, :], start=(st == 0), stop=(st == N_ST - 1))
                    # row-sum accumulation on the DVE (off the PE)
                    if st == 0:
                        nc.vector.tensor_copy(l_sb[:], pT[:])
                    else:
                        nc.vector.tensor_add(l_sb[:], l_sb[:], pT[:])

                if qb + 1 < N_QB:
                    qT_cur = emit_qproj(qb + 1)

                # --- epilogue: 1/l arranged with queries on partitions.
                # One fp32 matmul per 128-query block folds the partition sum
                # AND the transpose: lt[q, 0] = sum_p l_sb[p, qt*128 + q]
                # (lhsT = l_sb slice, rhs = ones column) ---
                lt_ps = ps_lp.tile([128, 4], F32, tag="l", name=f"lt{qb}")
                for qt in range(4):
                    nc.tensor.matmul(lt_ps[:, qt : qt + 1],
                                     l_sb[:, qt * 128 : (qt + 1) * 128],
                                     ones_f32[:, 0:1])
                rlT = rlp.tile([128, 4], F32, tag="rlT", name=f"rlT{qb}")
                nc.vector.reciprocal(rlT[:], lt_ps[:])

                # out rows = at * (1/l); for the final block split the scaled
                # copies across Vector+Scalar and both DMA queues to shorten
                # the kernel tail
                last = qb == N_QB - 1
                for qt in range(4):
                    ot = osbp.tile([128, D], F32, tag="ot", name=f"ot{qb}_{qt}")
                    row = out[(qb * 4 + qt) * 128 : (qb * 4 + qt + 1) * 128, :]
                    if last:
                        nc.vector.tensor_scalar_mul(ot[:, 0:256], at_ps[qt][:, 0:256],
                                                    rlT[:, qt : qt + 1])
                        nc.scalar.activation(ot[:, 256:512], at_ps[qt][:, 256:512],
                                             ActFn.Copy, scale=rlT[:, qt : qt + 1])
                        nc.sync.dma_start(row[:, 0:256], ot[:, 0:256])
                        nc.scalar.dma_start(row[:, 256:512], ot[:, 256:512])
                    else:
                        nc.vector.tensor_scalar_mul(ot[:], at_ps[qt][:], rlT[:, qt : qt + 1])
                        eng = nc.sync if qt % 2 == 0 else nc.scalar
                        eng.dma_start(row, ot[:])

        const.release()
        persist.release()

    nc.compile()
    return nc


_NC_CACHE = None


def _get_nc():
    global _NC_CACHE
    if _NC_CACHE is None:
        _NC_CACHE = _build_program()
    return _NC_CACHE


def kernel(**inputs):
    f32 = np.float32
    x = np.asarray(inputs["x"], dtype=f32)
    # host-side transpose to xT[c, s] per batch, cast bf16
    xt = np.ascontiguousarray(
        x.reshape(B, S, C).transpose(0, 2, 1)).astype(ml_dtypes.bfloat16)

    def tile_x(xb):
        # xT[c, s] -> [rg, dc, p, s] so each (rg, dc) chunk is contiguous
        return np.ascontiguousarray(
            xb.reshape(N_DC, 128, N_RG, 512).transpose(2, 0, 1, 3))

    Wq = np.asarray(inputs["Wq"], dtype=f32)
    Wk = np.asarray(inputs["Wk"], dtype=f32)
    Wv = np.asarray(inputs["Wv"], dtype=f32)
    Wo = np.asarray(inputs["Wo"], dtype=f32)
    # host-side weight folds (exact algebra, f32):
    #   score = [x (Wq Wk^T) + bq Wk^T] x^T  (+ per-query const, cancels)
    #   out   = (P/l) x (Wv Wo) + (bv Wo + bo)
    M = np.ascontiguousarray(Wq @ Wk.T).astype(ml_dtypes.bfloat16)
    N = np.ascontiguousarray(Wv @ Wo).astype(ml_dtypes.bfloat16)
    bqf = np.ascontiguousarray(np.asarray(inputs["bq"], dtype=f32) @ Wk.T)
    bo_eff = np.asarray(inputs["bo"], dtype=f32) + np.asarray(inputs["bv"], dtype=f32) @ Wo

    in_maps = []
    for c in range(N_CORES):
        b, h = divmod(c, 2)
        xb = xt[b]
        if h:
            # roll keys so this core's query half occupies columns 0..2047;
            # attention over keys is order-invariant so this is exact.
            xb = np.concatenate([xb[:, Q:], xb[:, :Q]], axis=1)
        in_maps.append({"x": tile_x(xb), "bq": bqf, "M": M, "N": N})

    nc = _get_nc()
    try:
        res = run_bass_kernel_spmd(nc, in_maps, core_ids=list(range(N_CORES)))
    except Exception:
        # transient NRT/device hiccups recover on retry
        import time
        time.sleep(15)
        res = run_bass_kernel_spmd(nc, in_maps, core_ids=list(range(N_CORES)))

    out = np.empty((B, S, D), dtype=f32)
    for c in range(N_CORES):
        b, h = divmod(c, 2)
        out[b, h * Q : (h + 1) * Q] = res.results[c]["out"]
    if np.any(bo_eff):
        out += bo_eff
    return out.reshape(B, 64, 64, D)
